# revision 3
# baseline (speedup 1.0000x reference)
"""Trainium2 Bass kernel for the 10-mode gate contraction (fp16).

y = transpose_back(einsum('ab...,ABab->AB...', transpose(x), B)) for
x of shape (6,)*10, gate wires [2, 5], B of shape (6, 6, 6, 6).

The op is a pure streaming workload (each element is touched once), so
HBM traffic is the roofline.  Design:

- Host casts x / B / y to fp16 (PSUM accumulates fp32; max rel err
  ~7e-4, far inside the 2e-2 gate) halving DMA bytes vs f32.
- Host re-lays x out as [p=(m0,m1), a=m2, b=m5, q=(m3,m4), r=(m6..m9)]
  and shards r across the 8 cores, so per core x is [36, 36, 5832]
  fp16 with the gate modes (a,b) packed INSIDE each p-slice: every
  3-p-slice block is one contiguous [108, 5832] region and both the
  load and store DMAs are flat access patterns.
- Each DMA moves 2 such blocks ([108, 2, 5832], 2.5 MB, 6 in + 6 out
  per core) to amortize per-DMA fixed cost; in-DMAs issue from the SP
  (sync) HWDGE ring, out-DMAs from the ACT (scalar) ring.
- The gate itself is a block-diagonal 108x108 fp16 matmul (blocks
  delta_gg' B^T, 3 p-slices per matmul, 486-column PSUM chunks); the
  PSUM->SBUF fp32->fp16 copies alternate DVE / ACT so no engine queue
  serializes the DMA pipeline.

Measured ~102 us/core on HW (chained-execution bench), ~4.3x the f32
baseline; consistent with the ~308 GB/s effective DMA bandwidth +
0.35 us/DMA observed across tilings.
"""

import sys
from contextlib import nullcontext

sys.path.insert(0, "/opt/trn_rl_repo")

import numpy as np

NCORES = 8
C = 6
NP, NAB, NQ, NR = 36, 36, 36, 1296
RS = NR // NCORES
QR = NQ * RS
NCHUNK = 486
GROUP = 3
PAIR = 2                    # p-groups per DMA tile

_compiled = None


def _build_reps(reps=None):
    import concourse.bacc as bacc
    import concourse.mybir as mybir
    import concourse.tile as tile

    DT = mybir.dt.float16
    DT32 = mybir.dt.float32
    nc = bacc.Bacc("TRN2", target_bir_lowering=False, debug=False,
                   num_devices=NCORES)
    x_in = nc.dram_tensor("x", [NP, NAB, QR], DT, kind="ExternalInput")
    w_in = nc.dram_tensor("w", [108, 108], DT, kind="ExternalInput")
    y_out = nc.dram_tensor("y", [NP, NAB, QR], DT, kind="ExternalOutput")

    with tile.TileContext(nc) as tc:
        with (
            tc.tile_pool(name="wpool", bufs=1) as wpool,
            tc.tile_pool(name="inpool", bufs=3) as inpool,
            tc.tile_pool(name="outpool", bufs=3) as outpool,
            tc.tile_pool(name="psum", bufs=8, space="PSUM") as psum_pool,
        ):
            wtile = wpool.tile([108, 108], DT)
            nc.sync.dma_start(out=wtile[:, :], in_=w_in.ap())

            loop = (tc.For_i(0, reps, 1, hint_engines=(mybir.EngineType.PE,))
                    if reps is not None else nullcontext())
            with loop:
                for p0 in range(0, NP, GROUP * PAIR):
                    xt = inpool.tile([108, PAIR, QR], DT)
                    src = x_in.ap()[p0:p0 + GROUP * PAIR].rearrange(
                        "(h g) ab qr -> (g ab) h qr", h=PAIR)
                    nc.sync.dma_start(out=xt[:, :, :], in_=src)

                    ot = outpool.tile([108, PAIR, QR], DT)
                    for h in range(PAIR):
                        for i, c in enumerate(range(0, QR, NCHUNK)):
                            ps = psum_pool.tile([108, NCHUNK], DT32)
                            nc.tensor.matmul(out=ps[:, :], lhsT=wtile[:, :],
                                             rhs=xt[:, h, c:c + NCHUNK],
                                             start=True, stop=True)
                            if i % 2 == 0:
                                nc.vector.tensor_copy(
                                    out=ot[:, h, c:c + NCHUNK], in_=ps[:, :])
                            else:
                                nc.scalar.copy(
                                    out=ot[:, h, c:c + NCHUNK], in_=ps[:, :])

                    dst = y_out.ap()[p0:p0 + GROUP * PAIR].rearrange(
                        "(h g) ab qr -> (g ab) h qr", h=PAIR)
                    nc.scalar.dma_start(out=dst, in_=ot[:, :, :])

    nc.compile()
    return nc


def _build():
    global _compiled
    if _compiled is None:
        _compiled = _build_reps(None)
    return _compiled


_PERM = (0, 1, 2, 5, 3, 4, 6, 7, 8, 9)
_INV_PERM = (0, 1, 2, 4, 5, 3, 6, 7, 8, 9)


def _prep_weights(B):
    Bm = np.ascontiguousarray(np.asarray(B), dtype=np.float32).reshape(36, 36)
    W = np.zeros((108, 108), np.float32)
    W4 = W.reshape(GROUP, 36, GROUP, 36)
    BmT = Bm.T.copy()
    for g in range(GROUP):
        W4[g, :, g, :] = BmT
    return W.astype(np.float16)


def _in_maps(x, B):
    W = _prep_weights(B)
    xv = np.asarray(x).reshape((C,) * 10).transpose(_PERM).astype(
        np.float16).reshape(NP, NAB, NQ, NR)
    return [
        {"x": np.ascontiguousarray(
            xv[..., k * RS:(k + 1) * RS]).reshape(NP, NAB, QR), "w": W}
        for k in range(NCORES)
    ]


def _gather(results):
    yp = np.empty((NP, NAB, NQ, NR), np.float16)
    for k in range(NCORES):
        yp[..., k * RS:(k + 1) * RS] = np.asarray(
            results[k]["y"]).reshape(NP, NAB, NQ, RS)
    return np.ascontiguousarray(
        yp.reshape((C,) * 10).transpose(_INV_PERM).astype(np.float32))


def _run(x, B, trace=False, **kwargs):
    from concourse.bass_utils import run_bass_kernel_spmd

    nc = _build()
    res = run_bass_kernel_spmd(nc, _in_maps(x, B), list(range(NCORES)),
                               trace=trace, **kwargs)
    return _gather(res.results), res


def kernel(x, B):
    y, _ = _run(x, B)
    return y
